# revision 26
# baseline (speedup 1.0000x reference)
"""
Distributed GQA attention block for Trainium2 (8 NeuronCores).

Problem: out = AttentionBlock(x; wq, wk, wv, wo)
  B=2, S=2048, DIM=4096, n_heads=32, n_kv_heads=8, head_dim=128,
  rope theta=5e5, causal, softmax, f32 I/O.

Sharding (tensor-parallel over heads, pipelined ReduceScatter after a
ROW-parallel output projection): core c owns 4 query heads and 1 kv head;
attention runs per-core over the full sequence; the output projection is
row-sharded and partial products are ReduceScattered over token groups.

Matmul precision scheme (hi/lo double-fp8 on the Tensor engine):
  The wq/wk/wv and wo contractions run as fp8(e4m3) DoubleRow matmuls with
  a hi+lo operand decomposition: A ~= A_hi + A_lo (both e4m3). Per 256-row
  contraction super-tile this takes 3 DoubleRow instructions
     hi.hi (two k-tiles paired in the DR slots)
     cross(kt): (w_hi x_lo + w_lo x_hi) packed in one DR via its 2 slots
  at 0.5 cyc/row each = 1.5 cyc vs bf16's 2.0 -> 25% less PE time, with
  BETTER precision than bf16 (~0.12% vs 0.23% rel).
  Scales keep every fp8 tensor in e4m3's sweet spot: x at natural scale,
  weights x64 (sigma 1.28), attention output x4 (ones-column = 16 with v
  held at 64x), wo x64; the net 256x output scale is removed for free in
  the PSUM->SBUF output copies (scale 1/256).
  Scores + probs + PV stay bf16: single-fp8 anywhere in the softmax path
  measurably fails the 2e-2 gate (score noise passes through undamped).

RoPE runs in bf16 (DVE 4x mode) with 1/64 folded into the cos/sin tables
to undo the 64x weight scale.
"""

import math
from types import SimpleNamespace

import numpy as np
import ml_dtypes

P = 128
BF16 = ml_dtypes.bfloat16
F8 = ml_dtypes.float8_e4m3

_CACHE = {}
_TRACE = False


def make_cfg(B=2, S=2048, DIM=4096, H=32, KVH=8, HD=128, THETA=500000.0,
             NCORES=8):
    c = SimpleNamespace(B=B, S=S, DIM=DIM, H=H, KVH=KVH, HD=HD, THETA=THETA,
                        NCORES=NCORES)
    c.T = B * S
    c.HPC = H // NCORES          # query heads per core
    c.QF = c.HPC * HD            # query features per core
    c.SCALE = 1.0 / math.sqrt(HD)
    c.TCH = 512                  # token chunk
    c.NKT = DIM // P             # contraction tiles
    c.NTT = c.T // P             # token tiles
    c.NCH = c.T // c.TCH         # token chunks
    c.SQT = S // P               # q/k tiles per sequence
    c.VW = HD + 1                # v + ones column
    # ReduceScatter groups (token counts): batch 1's groups are larger so
    # the 15us-constant collective chain keeps up with the tail-end wo rate
    c.RSG = [[512, 512, 512, 512], [768, 768, 512]]
    assert all(sum(gs) == S for gs in c.RSG)
    c.OSH = c.TCH // NCORES      # output token rows per core per group
    assert S % c.TCH == 0 and c.T % c.TCH == 0 and DIM % P == 0
    assert KVH == NCORES and c.HPC == H // KVH
    return c


def _build_graph(c, phases=4):
    """Build + compile the SPMD Bass graph (same program on every core)."""
    import concourse.mybir as mybir
    import concourse.tile as tile
    from concourse import bacc

    fp32 = mybir.dt.float32
    bf16 = mybir.dt.bfloat16
    fp8 = mybir.dt.float8e4
    DR = mybir.MatmulPerfMode.DoubleRow

    nc = bacc.Bacc(
        "TRN2",
        target_bir_lowering=False,
        debug=False,
        enable_asserts=True,
        num_devices=c.NCORES,
    )

    # ---- kernel I/O ----
    # hi/lo fp8 operand streams, rows ordered (kt, slot, p):
    #   x8  slots (lo, hi); w*8 slots (hi, lo)  [see cross-term pairing]
    x8 = nc.dram_tensor("x8", [c.NKT * 2 * P, c.T], fp8,
                        kind="ExternalInput").ap()
    wq8 = nc.dram_tensor("wq8", [c.NKT * 2 * P, c.QF], fp8,
                         kind="ExternalInput").ap()
    wk8 = nc.dram_tensor("wk8", [c.NKT * 2 * P, c.HD], fp8,
                         kind="ExternalInput").ap()
    wv8 = nc.dram_tensor("wv8", [c.NKT * 2 * P, c.HD], fp8,
                         kind="ExternalInput").ap()
    # wo8 rows (ft, slot(lo,hi), p)
    wo8 = nc.dram_tensor("wo8", [c.HPC * 2 * P, c.DIM], fp8,
                         kind="ExternalInput").ap()
    cosi = nc.dram_tensor("cosi", [P, c.T], bf16, kind="ExternalInput").ap()
    sini = nc.dram_tensor("sini", [P, c.T], bf16, kind="ExternalInput").ap()
    tril = nc.dram_tensor("tril", [P, P], bf16, kind="ExternalInput").ap()
    ident = nc.dram_tensor("ident", [P, P], bf16, kind="ExternalInput").ap()
    out = nc.dram_tensor("out", [c.NCH * c.OSH, c.DIM], bf16,
                         kind="ExternalOutput").ap()

    Exp = mybir.ActivationFunctionType.Exp
    Copy = mybir.ActivationFunctionType.Copy
    Mult = mybir.AluOpType.mult
    Sub = mybir.AluOpType.subtract
    TPP = c.TCH // P          # token sub-tiles per chunk
    NQT = c.HPC + 1           # rope targets per chunk: HPC q tiles + 1 k
    SPB = c.S // P            # 128-token tiles per batch
    CPB = c.NCH // c.B        # token chunks per batch
    KG = 4                    # contraction k-tiles fetched per DMA
    OSCL = 1.0 / 256.0        # output descale (4x attn * 64x wo)

    with tile.TileContext(nc) as tc:
        # ------- static SBUF tensors (split per batch) -------
        qT_b, kT_b, v_b, free_stat = [], [], [], []
        for b in range(c.B):
            t_, f_ = tc.tile([P, c.HPC, c.S], bf16, name=f"qT_sb{b}")
            qT_b.append(t_); free_stat.append(f_)
            t_, f_ = tc.tile([P, c.S], bf16, name=f"kT_sb{b}")
            kT_b.append(t_); free_stat.append(f_)
            t_, f_ = tc.tile([P, SPB, c.VW], bf16, name=f"v_sb{b}")
            v_b.append(t_); free_stat.append(f_)
        tril_sb, free_tril = tc.tile([P, P], bf16, name="tril_sb")
        idb_sb, free_id = tc.tile([P, P], bf16, name="idb_sb")
        c256_sb, free_c256 = tc.tile([P, 1], fp32, name="c256_sb")
        free_stat += [free_tril, free_id, free_c256]

        for b in range(c.B):
            # v is stored at 64x true scale; ones-col 16 => ao = 4*attn
            nc.vector.memset(v_b[b][:, :, c.HD:c.VW], 16.0)
        nc.vector.memset(c256_sb[:], OSCL)

        # dummy exp at t=0 pulls the ACT-table load off the critical path
        warm_sb, free_warm = tc.tile([1, 1], fp32, name="warm_sb")
        nc.scalar.activation(warm_sb[:], v_b[0][0:1, 0, c.HD:c.HD + 1], Exp)
        free_stat.append(free_warm)

        # flattened RS groups: (batch, local tok start, tok count, out row 0)
        rs_groups = []
        orow = 0
        for b in range(c.B):
            lt = 0
            for gsz in c.RSG[b]:
                rs_groups.append((b, lt, gsz, orow))
                lt += gsz
                orow += gsz // c.NCORES

        with tc.tile_pool(name="dram", bufs=1, space="DRAM") as dramp:
            part_g = [
                dramp.tile([gsz, c.DIM], bf16, name=f"part{g}")
                for g, (_, _, gsz, _) in enumerate(rs_groups)
            ]
            rs_g = [
                dramp.tile([gsz // c.NCORES, c.DIM], bf16, name=f"rs{g}")
                for g, (_, _, gsz, _) in enumerate(rs_groups)
            ]

            # ============ Phase 1: projections + RoPE ============
            WG = 8                    # wk/wv k-tiles per DMA
            with tc.tile_pool(name="wpool", bufs=1) as wpool, \
                 tc.tile_pool(name="xpool", bufs=3) as xpool, \
                 tc.tile_pool(name="tabs", bufs=2) as tabs, \
                 tc.tile_pool(name="rope", bufs=2) as ropep, \
                 tc.tile_pool(name="pj_ps", bufs=1, space="PSUM") as pjps:

                # k/v weights ride the gpsimd queue in WG-tile mega-DMAs
                wqb = [None] * (c.NKT // KG)
                wkb, wvb = [], []
                for gi in range(c.NKT // WG):
                    r0 = gi * WG * 2 * P
                    r1 = (gi + 1) * WG * 2 * P
                    wkt = wpool.tile([P, WG, 2, c.HD], fp8, tag="wk",
                                     bufs=c.NKT // WG, name=f"wk_g{gi}")
                    wvt = wpool.tile([P, WG, 2, c.HD], fp8, tag="wv",
                                     bufs=c.NKT // WG, name=f"wv_g{gi}")
                    if gi == 0:
                        # split head: first 2 k-tiles land fast
                        rm = r0 + 2 * 2 * P
                        nc.gpsimd.dma_start(
                            wkt[:, 0:2, :, :],
                            wk8[r0:rm, :].rearrange(
                                "(g i p) h -> p g i h", p=P, i=2))
                        nc.gpsimd.dma_start(
                            wvt[:, 0:2, :, :],
                            wv8[r0:rm, :].rearrange(
                                "(g i p) h -> p g i h", p=P, i=2))
                        nc.gpsimd.dma_start(
                            wkt[:, 2:WG, :, :],
                            wk8[rm:r1, :].rearrange(
                                "(g i p) h -> p g i h", p=P, i=2))
                        nc.gpsimd.dma_start(
                            wvt[:, 2:WG, :, :],
                            wv8[rm:r1, :].rearrange(
                                "(g i p) h -> p g i h", p=P, i=2))
                    else:
                        nc.gpsimd.dma_start(
                            wkt[:],
                            wk8[r0:r1, :].rearrange(
                                "(g i p) h -> p g i h", p=P, i=2))
                        nc.gpsimd.dma_start(
                            wvt[:],
                            wv8[r0:r1, :].rearrange(
                                "(g i p) h -> p g i h", p=P, i=2))
                    wkb.append(wkt)
                    wvb.append(wvt)

                def wk_at(kt):
                    return wkb[kt // WG][:, kt % WG, :, :]     # [P,2,HD] (h,l)

                def wv_at(kt):
                    return wvb[kt // WG][:, kt % WG, :, :]

                def wk_hh(kt):  # [P,2,HD] hi slots of (kt, kt+1)
                    g, l0 = kt // WG, kt % WG
                    return wkb[g][:, l0:l0 + 2, 0, :]

                def wv_hh(kt):
                    g, l0 = kt // WG, kt % WG
                    return wvb[g][:, l0:l0 + 2, 0, :]

                def load_wq(kg):
                    wqt = wpool.tile([P, KG, 2, c.QF], fp8, tag="wq",
                                     bufs=c.NKT // KG, name=f"wq_g{kg}")
                    r0 = kg * KG * 2 * P
                    if kg == 0:
                        rm = r0 + 2 * P
                        nc.sync.dma_start(
                            wqt[:, 0:1, :, :],
                            wq8[r0:rm, :].rearrange(
                                "(g i p) f -> p g i f", p=P, i=2))
                        nc.sync.dma_start(
                            wqt[:, 1:KG, :, :],
                            wq8[rm:r0 + KG * 2 * P, :].rearrange(
                                "(g i p) f -> p g i f", p=P, i=2))
                    else:
                        nc.sync.dma_start(
                            wqt[:],
                            wq8[r0:r0 + KG * 2 * P, :].rearrange(
                                "(g i p) f -> p g i f", p=P, i=2))
                    wqb[kg] = wqt

                for ch in range(c.NCH):
                    t0 = ch * c.TCH
                    bch = ch // CPB           # batch of this chunk
                    lt0 = t0 - bch * c.S      # batch-local token offset
                    q_ps = [
                        pjps.tile([P, c.TCH], fp32, tag=f"q{ft}", bufs=1,
                                  name=f"q_ps{ft}")
                        for ft in range(c.HPC)
                    ]
                    k_ps = pjps.tile([P, c.TCH], fp32, tag="k", bufs=1)
                    v_ps = pjps.tile([P, TPP, P], fp32, tag="v", bufs=1)

                    for kg in range(c.NKT // KG):
                        # one DMA brings KG k-tiles x (lo,hi) = 1MB fp8
                        xt = xpool.tile([P, KG, 2, c.TCH], fp8, tag="xt")
                        r0 = kg * KG * 2 * P
                        if ch == 0 and kg == 0:
                            nc.sync.dma_start(
                                xt[:, 0:1, :, :],
                                x8[r0:r0 + 2 * P, t0:t0 + c.TCH].rearrange(
                                    "(g i p) t -> p g i t", p=P, i=2))
                            nc.sync.dma_start(
                                xt[:, 1:KG, :, :],
                                x8[r0 + 2 * P:r0 + KG * 2 * P,
                                   t0:t0 + c.TCH].rearrange(
                                    "(g i p) t -> p g i t", p=P, i=2))
                        else:
                            nc.sync.dma_start(
                                xt[:],
                                x8[r0:r0 + KG * 2 * P,
                                   t0:t0 + c.TCH].rearrange(
                                    "(g i p) t -> p g i t", p=P, i=2))
                        if ch == 0:
                            load_wq(kg)
                        if ch == 0 and kg == 0:
                            nc.sync.dma_start(tril_sb[:], tril[:])
                            nc.sync.dma_start(idb_sb[:], ident[:])
                        wqt = wqb[kg]
                        for a2 in range(0, KG, 2):
                            kt = kg * KG + a2
                            st = kt == 0
                            sp = kt + 1 == c.NKT - 1

                            def mm_kv():
                                # cross(kt) first: at t=0 it only needs the
                                # small split-head DMA (kt alone), while the
                                # hi.hi pair also waits on kt+1
                                nc.tensor.matmul(
                                    k_ps[:], lhsT=wk_at(kt),
                                    rhs=xt[:, a2, :, :],
                                    start=st, stop=False, perf_mode=DR)
                                nc.tensor.matmul(
                                    k_ps[:], lhsT=wk_hh(kt),
                                    rhs=xt[:, a2:a2 + 2, 1, :],
                                    start=False, stop=False, perf_mode=DR)
                                nc.tensor.matmul(
                                    k_ps[:], lhsT=wk_at(kt + 1),
                                    rhs=xt[:, a2 + 1, :, :],
                                    start=False, stop=sp, perf_mode=DR)
                                # v token-major: x is lhsT
                                for sub in range(TPP):
                                    s0 = sub * P
                                    nc.tensor.matmul(
                                        v_ps[:, sub, :],
                                        lhsT=xt[:, a2, :, s0:s0 + P],
                                        rhs=wv_at(kt),
                                        start=(st and sub == 0), stop=False,
                                        perf_mode=DR)
                                    nc.tensor.matmul(
                                        v_ps[:, sub, :],
                                        lhsT=xt[:, a2:a2 + 2, 1, s0:s0 + P],
                                        rhs=wv_hh(kt),
                                        start=False, stop=False,
                                        perf_mode=DR)
                                    nc.tensor.matmul(
                                        v_ps[:, sub, :],
                                        lhsT=xt[:, a2 + 1, :, s0:s0 + P],
                                        rhs=wv_at(kt + 1),
                                        start=False, stop=sp,
                                        perf_mode=DR)

                            if ch == 0:
                                mm_kv()
                            for ft in range(c.HPC):
                                f0 = ft * P
                                nc.tensor.matmul(
                                    q_ps[ft][:],
                                    lhsT=wqt[:, a2, :, f0:f0 + P],
                                    rhs=xt[:, a2, :, :],
                                    start=st, stop=False, perf_mode=DR)
                                nc.tensor.matmul(
                                    q_ps[ft][:],
                                    lhsT=wqt[:, a2:a2 + 2, 0, f0:f0 + P],
                                    rhs=xt[:, a2:a2 + 2, 1, :],
                                    start=False, stop=False, perf_mode=DR)
                                nc.tensor.matmul(
                                    q_ps[ft][:],
                                    lhsT=wqt[:, a2 + 1, :, f0:f0 + P],
                                    rhs=xt[:, a2 + 1, :, :],
                                    start=False, stop=sp, perf_mode=DR)
                            if ch != 0:
                                mm_kv()

                    # ---- RoPE on all q tiles + k at once (bf16, DVE 4x) ----
                    ct = tabs.tile([P, c.TCH], bf16, tag="cos")
                    st_t = tabs.tile([P, c.TCH], bf16, tag="sin")
                    nc.sync.dma_start(ct[:], cosi[:, t0:t0 + c.TCH])
                    nc.sync.dma_start(st_t[:], sini[:, t0:t0 + c.TCH])

                    qbig = ropep.tile([P, NQT, c.TCH], bf16, tag="qbig",
                                      name="qbig")
                    gt0 = lt0 // P
                    nc.scalar.activation(qbig[:, 0, :], q_ps[0][:], Copy)
                    nc.vector.tensor_copy(qbig[:, 1, :], q_ps[1][:])
                    nc.scalar.activation(qbig[:, 2, :], q_ps[2][:], Copy)
                    nc.vector.tensor_copy(qbig[:, 3, :], q_ps[3][:])
                    nc.scalar.activation(qbig[:, c.HPC, :], k_ps[:], Copy)
                    nc.scalar.activation(v_b[bch][:, gt0, 0:c.HD],
                                         v_ps[:, 0, :], Copy)
                    nc.vector.tensor_copy(v_b[bch][:, gt0 + 1, 0:c.HD],
                                          v_ps[:, 1, :])
                    nc.scalar.activation(v_b[bch][:, gt0 + 2, 0:c.HD],
                                         v_ps[:, 2, :], Copy)
                    nc.vector.tensor_copy(v_b[bch][:, gt0 + 3, 0:c.HD],
                                          v_ps[:, 3, :])

                    qsw = ropep.tile([P, NQT, c.TCH], bf16, tag="qsw",
                                     name="qsw")
                    # pair swap == half-partition block swap (even|odd split)
                    nc.sync.dma_start(qsw[0:64, :, :], qbig[64:128, :, :])
                    nc.sync.dma_start(qsw[64:128, :, :], qbig[0:64, :, :])

                    ctb = ct[:, None, :].to_broadcast((P, NQT, c.TCH))
                    stb = st_t[:, None, :].to_broadcast((P, NQT, c.TCH))
                    eng = nc.vector
                    eng.tensor_mul(qbig[:], qbig[:], ctb)
                    eng.tensor_mul(qsw[:], qsw[:], stb)
                    rr = ropep.tile([P, NQT, c.TCH], bf16, tag="rr", name="rr")
                    eng.tensor_add(rr[:], qbig[:], qsw[:])
                    for ft in range(c.HPC):
                        eng.tensor_copy(
                            qT_b[bch][:, ft, lt0:lt0 + c.TCH], rr[:, ft, :])
                    eng.tensor_copy(
                        kT_b[bch][:, lt0:lt0 + c.TCH], rr[:, c.HPC, :])

            # ====== Phases 2+4 per batch: attention -> wo -> RS ======
            if phases >= 2:
                with tc.tile_pool(name="spool", bufs=3, space="PSUM") as spool, \
                     tc.tile_pool(name="opool", bufs=4, space="PSUM") as opool, \
                     tc.tile_pool(name="tpool", bufs=1, space="PSUM") as tpool, \
                     tc.tile_pool(name="ppool", bufs=32) as ppool, \
                     tc.tile_pool(name="apool", bufs=4) as apool, \
                     tc.tile_pool(name="wop", bufs=1) as wop, \
                     tc.tile_pool(name="atp", bufs=2) as atp, \
                     tc.tile_pool(name="obp", bufs=2) as obp:

                    # wo row-shard as one [P, HPC, 2(lo,hi), DIM] fp8 tile
                    wo_sb = wop.tile([P, c.HPC, 2, c.DIM], fp8, name="wo_sb")
                    for f in range(c.HPC):
                        nc.gpsimd.dma_start(
                            wo_sb[:, f, :, :],
                            wo8[f * 2 * P:(f + 1) * 2 * P, :].rearrange(
                                "(i p) d -> p i d", p=P))

                    def finalize(pend):
                        # bf16 PE transpose of the normalized attention tile,
                        # then split hi/lo fp8 into the attnT store
                        # (layout [P, ft, (h,l), S])
                        p_attnT, p_h, p_qi, p_ao = pend
                        tp = tpool.tile([P, P], bf16, tag="tp", name="tp")
                        nc.tensor.transpose(tp[:], p_ao[:], idb_sb[:])
                        t0c = p_qi * P
                        hi_sl = p_attnT[:, p_h, 0, t0c:t0c + P]
                        nc.vector.tensor_copy(hi_sl, tp[:])
                        nc.vector.tensor_sub(
                            p_attnT[:, p_h, 1, t0c:t0c + P], tp[:], hi_sl)

                    def emit_last_pv(pend):
                        # deferred final PV steps + normalize (ao = 4*attn)
                        p_ops, p_steps, p_start, p_qi, p_attnT, p_h = pend
                        for i, (p_pt, p_off, p_vt) in enumerate(p_steps):
                            nc.tensor.matmul(
                                p_ops, lhsT=p_pt[:, p_off:p_off + P],
                                rhs=p_vt,
                                start=(p_start and i == 0),
                                stop=(i == len(p_steps) - 1),
                            )
                        rec = apool.tile([P, 1], fp32, tag="rec", name="rec")
                        nc.vector.reciprocal(rec[:], p_ops[:, c.HD:c.VW])
                        ao = apool.tile([P, P], bf16, tag="ao", name="ao")
                        nc.vector.tensor_scalar_mul(
                            ao[:], p_ops[:, 0:c.HD], rec[:])
                        return (p_attnT, p_h, p_qi, ao)

                    pending_o = None
                    pend_pv = None
                    o_pp = opool.tile([P, 2, c.VW], fp32, tag="o", bufs=1,
                                      name="o_pp")
                    n_ki = 0
                    SC = 1024             # score tile width (one exp per tile)

                    def score_tile(b, qh, kj, c0, w, tril_m):
                        s_ps = spool.tile([P, SC], fp32, tag="s",
                                          name="s_ps")
                        for sub in range(0, w, c.TCH):
                            ws = min(c.TCH, w - sub)
                            nc.tensor.matmul(
                                s_ps[:, sub:sub + ws],
                                lhsT=kT_b[b][:, kj * P:(kj + 1) * P],
                                rhs=qh[:, c0 + sub:c0 + sub + ws],
                                start=True, stop=True,
                            )
                        pt = ppool.tile([P, SC], bf16, tag="pt", bufs=28,
                                        name="pt")
                        nc.scalar.activation(pt[:, :w], s_ps[:, :w], Exp,
                                             scale=c.SCALE)
                        if tril_m:
                            nc.vector.tensor_mul(pt[:, 0:P], pt[:, 0:P],
                                                 tril_sb[:])
                        return pt

                    def emit_unit(b, h, ki, attnT, pts):
                        nonlocal pending_o, pend_pv, n_ki
                        qh = qT_b[b][:, h, :]
                        q0 = ki * P
                        nmc = (c.S - q0 + SC - 1) // SC
                        for mc in range(nmc):
                            c0 = q0 + mc * SC
                            pts[(ki, mc)] = score_tile(
                                b, qh, ki, c0, min(SC, c.S - c0), mc == 0)
                        if pend_pv is not None:
                            old_fin = pending_o
                            pending_o = emit_last_pv(pend_pv)
                            pend_pv = None
                            if old_fin is not None:
                                finalize(old_fin)
                        qi = ki
                        o_ps = o_pp[:, n_ki % 2, :]
                        n_ki += 1
                        for kj in range(qi):
                            qoff = (qi - kj) * P
                            mc = qoff // SC
                            off = qoff % SC
                            nc.tensor.matmul(
                                o_ps,
                                lhsT=pts[(kj, mc)][:, off:off + P],
                                rhs=v_b[b][:, kj, :],
                                start=(kj == 0), stop=False,
                            )
                        pend_pv = (o_ps,
                                   [(pts[(qi, 0)], 0, v_b[b][:, qi, :])],
                                   qi == 0, qi, attnT, h)

                    attnTs = [
                        atp.tile([P, c.HPC, 2, c.S], fp8, tag="attnT",
                                 name=f"attnT{bb}")
                        for bb in range(c.B)
                    ]
                    pts_next = {}
                    pts_next2 = {}
                    pts_next3 = {}

                    def p4_batch(b, attnT, extra):
                        # row-parallel wo (hi/lo fp8 DR) + pipelined RS
                        nonlocal pending_o, pend_pv
                        ei = 0
                        gids = [g for g, rg in enumerate(rs_groups)
                                if rg[0] == b]
                        for g in gids:
                            _, lt0g, gsz, or0 = rs_groups[g]
                            for tt in range(gsz // P):  # 128-token tiles
                                lt = lt0g + tt * P
                                ob = obp.tile([P, c.DIM], bf16, tag="ob",
                                              bufs=3, name="ob")
                                for cc in range(c.DIM // SC):
                                    o4 = spool.tile([P, SC], fp32,
                                                    tag="s", name="o4_ps")
                                    for half in range(SC // c.TCH):
                                        hb = half * c.TCH
                                        d0 = cc * SC + hb
                                        # hi.hi over ft pairs, then cross
                                        for fp_ in range(0, c.HPC, 2):
                                            nc.tensor.matmul(
                                                o4[:, hb:hb + c.TCH],
                                                lhsT=attnT[:, fp_:fp_ + 2, 0,
                                                           lt:lt + P],
                                                rhs=wo_sb[:, fp_:fp_ + 2, 1,
                                                          d0:d0 + c.TCH],
                                                start=(fp_ == 0), stop=False,
                                                perf_mode=DR)
                                        for f in range(c.HPC):
                                            nc.tensor.matmul(
                                                o4[:, hb:hb + c.TCH],
                                                lhsT=attnT[:, f, :,
                                                           lt:lt + P],
                                                rhs=wo_sb[:, f, :,
                                                          d0:d0 + c.TCH],
                                                start=False,
                                                stop=(f == c.HPC - 1),
                                                perf_mode=DR)
                                    if pend_pv is not None:
                                        old_fin = pending_o
                                        pending_o = emit_last_pv(pend_pv)
                                        pend_pv = None
                                        if old_fin is not None:
                                            finalize(old_fin)
                                    elif pending_o is not None:
                                        finalize(pending_o)
                                        pending_o = None
                                    dst = ob[:, cc * SC:(cc + 1) * SC]
                                    if cc % 2 == 0:
                                        nc.scalar.activation(dst, o4[:], Copy,
                                                             scale=OSCL)
                                    else:
                                        nc.vector.tensor_scalar_mul(
                                            dst, o4[:], c256_sb[:])
                                nc.gpsimd.dma_start(
                                    part_g[g][tt * P:(tt + 1) * P, :], ob[:]
                                )
                                for _ in range(3):
                                    if ei < len(extra):
                                        extra[ei]()
                                        ei += 1
                            if phases >= 3:
                                nc.gpsimd.collective_compute(
                                    "ReduceScatter",
                                    mybir.AluOpType.add,
                                    replica_groups=[list(range(c.NCORES))],
                                    ins=[part_g[g][:].opt()],
                                    outs=[rs_g[g][:].opt()],
                                )
                                nc.sync.dma_start(
                                    out[or0:or0 + gsz // c.NCORES, :],
                                    rs_g[g][:],
                                )
                        assert ei == len(extra)

                    # batch-0 attention, all 4 heads
                    for h in range(c.HPC):
                        pts = {}
                        for ki in range(c.SQT):
                            emit_unit(0, h, ki, attnTs[0], pts)
                    # batch-1's LAST head runs here (not between the two p4
                    # phases): p4(b1) then follows p4(b0) immediately, so its
                    # partials - and the tail ReduceScatter chain - start
                    # ~29us earlier. Heads 0-2 of batch 1 still ride inside
                    # p4(b0) as extras.
                    pts_h3 = {}
                    for ki in range(c.SQT):
                        emit_unit(1, 3, ki, attnTs[1], pts_h3)
                    extra = [
                        (lambda kk=kk, hh=hh, pp=pp: emit_unit(
                            1, hh, kk, attnTs[1], pp))
                        for hh, pp in ((0, pts_next), (1, pts_next2),
                                       (2, pts_next3))
                        for kk in range(c.SQT)
                    ]
                    p4_batch(0, attnTs[0], extra)
                    p4_batch(1, attnTs[1], [])

        for f_ in reversed(free_stat):
            f_()

    nc.compile()
    return nc


def _hilo(a):
    """e4m3 hi/lo decomposition of a float32 array."""
    hi = a.astype(F8)
    lo = (a - hi.astype(np.float32)).astype(F8)
    return hi, lo


def _host_inputs(c, x, wq, wk, wv, wo):
    """Shard + lay out the inputs for the cores."""
    xT = np.ascontiguousarray(x.reshape(c.T, c.DIM).T)    # [DIM, T] fp32

    # x8 rows (kt, slot(lo,hi), p)
    xh, xl = _hilo(xT)
    x8 = np.empty((c.NKT, 2, P, c.T), F8)
    x8[:, 0] = xl.reshape(c.NKT, P, c.T)
    x8[:, 1] = xh.reshape(c.NKT, P, c.T)
    x8 = np.ascontiguousarray(x8.reshape(c.NKT * 2 * P, c.T))

    # even/odd split permutation within each head (q and k only)
    perm_head = np.concatenate([np.arange(0, c.HD, 2), np.arange(1, c.HD, 2)])

    def permute_heads(w):  # w: [DIM, n*HD]
        nh = w.shape[1] // c.HD
        w = w.reshape(c.DIM, nh, c.HD)[:, :, perm_head]
        return np.ascontiguousarray(w.reshape(c.DIM, nh * c.HD))

    def pack_w(w):  # [DIM, F] fp32 -> rows (kt, slot(hi,lo), p)
        hi, lo = _hilo(64.0 * w)
        o = np.empty((c.NKT, 2, P, w.shape[1]), F8)
        o[:, 0] = hi.reshape(c.NKT, P, -1)
        o[:, 1] = lo.reshape(c.NKT, P, -1)
        return np.ascontiguousarray(o.reshape(c.NKT * 2 * P, w.shape[1]))

    wq_p = pack_w(permute_heads(wq).astype(np.float32))
    wk_p = pack_w(permute_heads(wk).astype(np.float32))
    wv_p = pack_w(wv.astype(np.float32))

    def pack_wo(w):  # [QF, DIM] -> rows (ft, slot(lo,hi), p)
        hi, lo = _hilo(64.0 * w)
        o = np.empty((c.HPC, 2, P, c.DIM), F8)
        o[:, 0] = lo.reshape(c.HPC, P, c.DIM)
        o[:, 1] = hi.reshape(c.HPC, P, c.DIM)
        return np.ascontiguousarray(o.reshape(c.HPC * 2 * P, c.DIM))

    # rope tables with 1/64 folded (undoes the 64x weight scale), bf16
    hh = c.HD // 2
    inv = 1.0 / (c.THETA ** (np.arange(0, c.HD, 2, dtype=np.float64) / c.HD))
    pos = (np.arange(c.T) % c.S).astype(np.float64)
    ang = inv[:, None] * pos[None, :]              # [64, T]
    cosv = (np.cos(ang) / 64.0).astype(np.float32)
    sinv = (np.sin(ang) / 64.0).astype(np.float32)
    cosi = np.concatenate([cosv, cosv], 0).astype(BF16)
    sini = np.concatenate([-sinv, sinv], 0).astype(BF16)
    assert hh * 2 == P

    trilm = np.ascontiguousarray(
        np.tril(np.ones((P, P), np.float32)).T
    ).astype(BF16)                                  # [k, q]: 1 iff k<=q
    identm = np.eye(P, dtype=np.float32).astype(BF16)

    KHC = c.KVH // c.NCORES  # kv heads per core (=1)
    in_maps = []
    for cc in range(c.NCORES):
        in_maps.append({
            "x8": x8,
            "wq8": np.ascontiguousarray(wq_p[:, cc * c.QF:(cc + 1) * c.QF]),
            "wk8": np.ascontiguousarray(
                wk_p[:, cc * KHC * c.HD:(cc * KHC + 1) * c.HD]),
            "wv8": np.ascontiguousarray(
                wv_p[:, cc * KHC * c.HD:(cc * KHC + 1) * c.HD]),
            "wo8": pack_wo(
                wo[cc * c.QF:(cc + 1) * c.QF, :].astype(np.float32)),
            "cosi": cosi,
            "sini": sini,
            "tril": trilm,
            "ident": identm,
        })
    return in_maps


def assemble(c, outs):
    """outs[r]: per-core token shards, rows grouped by RS group; within
    group (b, lt0, gsz) core r holds global tokens
    [b*S + lt0 + r*gsz/NC, +gsz/NC)."""
    outs = [np.asarray(o) for o in outs]
    full = np.empty((c.T, c.DIM), np.float32)
    orow = 0
    for b in range(c.B):
        lt = 0
        for gsz in c.RSG[b]:
            sh = gsz // c.NCORES
            for r in range(c.NCORES):
                t0 = b * c.S + lt + r * sh
                full[t0:t0 + sh] = outs[r][orow:orow + sh].astype(np.float32)
            lt += gsz
            orow += sh
    return full.reshape(c.B, c.S, c.DIM)


def kernel(x, wq, wk, wv, wo):
    from concourse import bass_utils

    if "nc" not in _CACHE:
        _CACHE["cfg"] = make_cfg()
        _CACHE["nc"] = _build_graph(_CACHE["cfg"])
    nc = _CACHE["nc"]
    c = _CACHE["cfg"]

    in_maps = _host_inputs(
        c, np.asarray(x), np.asarray(wq), np.asarray(wk),
        np.asarray(wv), np.asarray(wo),
    )
    res = bass_utils.run_bass_kernel_spmd(
        nc, in_maps, core_ids=list(range(c.NCORES)), trace=_TRACE
    )
    _CACHE["last_results"] = res
    outs = [res.results[i]["out"] for i in range(c.NCORES)]
    return assemble(c, outs)


# revision 27
# speedup vs baseline: 1.0068x; 1.0068x over previous
"""
Distributed GQA attention block for Trainium2 (8 NeuronCores).

Problem: out = AttentionBlock(x; wq, wk, wv, wo)
  B=2, S=2048, DIM=4096, n_heads=32, n_kv_heads=8, head_dim=128,
  rope theta=5e5, causal, softmax, f32 I/O.

Sharding (tensor-parallel over heads, pipelined ReduceScatter after a
ROW-parallel output projection): core c owns 4 query heads and 1 kv head;
attention runs per-core over the full sequence; the output projection is
row-sharded and partial products are ReduceScattered over token groups.

Matmul precision scheme (hi/lo double-fp8 on the Tensor engine):
  The wq/wk/wv and wo contractions run as fp8(e4m3) DoubleRow matmuls with
  a hi+lo operand decomposition: A ~= A_hi + A_lo (both e4m3). Per 256-row
  contraction super-tile this takes 3 DoubleRow instructions
     hi.hi (two k-tiles paired in the DR slots)
     cross(kt): (w_hi x_lo + w_lo x_hi) packed in one DR via its 2 slots
  at 0.5 cyc/row each = 1.5 cyc vs bf16's 2.0 -> 25% less PE time, with
  BETTER precision than bf16 (~0.12% vs 0.23% rel).
  Scales keep every fp8 tensor in e4m3's sweet spot: x at natural scale,
  weights x64 (sigma 1.28), attention output x4 (ones-column = 16 with v
  held at 64x), wo x64; the net 256x output scale is removed for free in
  the PSUM->SBUF output copies (scale 1/256).
  Scores + probs + PV stay bf16: single-fp8 anywhere in the softmax path
  measurably fails the 2e-2 gate (score noise passes through undamped).

RoPE runs in bf16 (DVE 4x mode) with 1/64 folded into the cos/sin tables
to undo the 64x weight scale.
"""

import math
from types import SimpleNamespace

import numpy as np
import ml_dtypes

P = 128
BF16 = ml_dtypes.bfloat16
F8 = ml_dtypes.float8_e4m3

_CACHE = {}
_TRACE = False


def make_cfg(B=2, S=2048, DIM=4096, H=32, KVH=8, HD=128, THETA=500000.0,
             NCORES=8):
    c = SimpleNamespace(B=B, S=S, DIM=DIM, H=H, KVH=KVH, HD=HD, THETA=THETA,
                        NCORES=NCORES)
    c.T = B * S
    c.HPC = H // NCORES          # query heads per core
    c.QF = c.HPC * HD            # query features per core
    c.SCALE = 1.0 / math.sqrt(HD)
    c.TCH = 512                  # token chunk
    c.NKT = DIM // P             # contraction tiles
    c.NTT = c.T // P             # token tiles
    c.NCH = c.T // c.TCH         # token chunks
    c.SQT = S // P               # q/k tiles per sequence
    c.VW = HD + 1                # v + ones column
    # ReduceScatter groups (token counts): batch 1's groups are larger so
    # the 15us-constant collective chain keeps up with the tail-end wo rate
    c.RSG = [[512, 512, 512, 512], [1024, 1024]]
    assert all(sum(gs) == S for gs in c.RSG)
    c.OSH = c.TCH // NCORES      # output token rows per core per group
    assert S % c.TCH == 0 and c.T % c.TCH == 0 and DIM % P == 0
    assert KVH == NCORES and c.HPC == H // KVH
    return c


def _build_graph(c, phases=4):
    """Build + compile the SPMD Bass graph (same program on every core)."""
    import concourse.mybir as mybir
    import concourse.tile as tile
    from concourse import bacc

    fp32 = mybir.dt.float32
    bf16 = mybir.dt.bfloat16
    fp8 = mybir.dt.float8e4
    DR = mybir.MatmulPerfMode.DoubleRow

    nc = bacc.Bacc(
        "TRN2",
        target_bir_lowering=False,
        debug=False,
        enable_asserts=True,
        num_devices=c.NCORES,
    )

    # ---- kernel I/O ----
    # hi/lo fp8 operand streams, rows ordered (kt, slot, p):
    #   x8  slots (lo, hi); w*8 slots (hi, lo)  [see cross-term pairing]
    x8 = nc.dram_tensor("x8", [c.NKT * 2 * P, c.T], fp8,
                        kind="ExternalInput").ap()
    wq8 = nc.dram_tensor("wq8", [c.NKT * 2 * P, c.QF], fp8,
                         kind="ExternalInput").ap()
    wk8 = nc.dram_tensor("wk8", [c.NKT * 2 * P, c.HD], fp8,
                         kind="ExternalInput").ap()
    wv8 = nc.dram_tensor("wv8", [c.NKT * 2 * P, c.HD], fp8,
                         kind="ExternalInput").ap()
    # wo8 rows (ft, slot(lo,hi), p)
    wo8 = nc.dram_tensor("wo8", [c.HPC * 2 * P, c.DIM], fp8,
                         kind="ExternalInput").ap()
    cosi = nc.dram_tensor("cosi", [P, c.T], bf16, kind="ExternalInput").ap()
    sini = nc.dram_tensor("sini", [P, c.T], bf16, kind="ExternalInput").ap()
    tril = nc.dram_tensor("tril", [P, P], bf16, kind="ExternalInput").ap()
    ident = nc.dram_tensor("ident", [P, P], bf16, kind="ExternalInput").ap()
    out = nc.dram_tensor("out", [c.NCH * c.OSH, c.DIM], bf16,
                         kind="ExternalOutput").ap()

    Exp = mybir.ActivationFunctionType.Exp
    Copy = mybir.ActivationFunctionType.Copy
    Mult = mybir.AluOpType.mult
    Sub = mybir.AluOpType.subtract
    TPP = c.TCH // P          # token sub-tiles per chunk
    NQT = c.HPC + 1           # rope targets per chunk: HPC q tiles + 1 k
    SPB = c.S // P            # 128-token tiles per batch
    CPB = c.NCH // c.B        # token chunks per batch
    KG = 4                    # contraction k-tiles fetched per DMA
    OSCL = 1.0 / 256.0        # output descale (4x attn * 64x wo)

    with tile.TileContext(nc) as tc:
        # ------- static SBUF tensors (split per batch) -------
        qT_b, kT_b, v_b, free_stat = [], [], [], []
        for b in range(c.B):
            t_, f_ = tc.tile([P, c.HPC, c.S], bf16, name=f"qT_sb{b}")
            qT_b.append(t_); free_stat.append(f_)
            t_, f_ = tc.tile([P, c.S], bf16, name=f"kT_sb{b}")
            kT_b.append(t_); free_stat.append(f_)
            t_, f_ = tc.tile([P, SPB, c.VW], bf16, name=f"v_sb{b}")
            v_b.append(t_); free_stat.append(f_)
        tril_sb, free_tril = tc.tile([P, P], bf16, name="tril_sb")
        idb_sb, free_id = tc.tile([P, P], bf16, name="idb_sb")
        c256_sb, free_c256 = tc.tile([P, 1], fp32, name="c256_sb")
        free_stat += [free_tril, free_id, free_c256]

        for b in range(c.B):
            # v is stored at 64x true scale; ones-col 16 => ao = 4*attn
            nc.vector.memset(v_b[b][:, :, c.HD:c.VW], 16.0)
        nc.vector.memset(c256_sb[:], OSCL)

        # dummy exp at t=0 pulls the ACT-table load off the critical path
        warm_sb, free_warm = tc.tile([1, 1], fp32, name="warm_sb")
        nc.scalar.activation(warm_sb[:], v_b[0][0:1, 0, c.HD:c.HD + 1], Exp)
        free_stat.append(free_warm)

        # flattened RS groups: (batch, local tok start, tok count, out row 0)
        rs_groups = []
        orow = 0
        for b in range(c.B):
            lt = 0
            for gsz in c.RSG[b]:
                rs_groups.append((b, lt, gsz, orow))
                lt += gsz
                orow += gsz // c.NCORES

        with tc.tile_pool(name="dram", bufs=1, space="DRAM") as dramp:
            part_g = [
                dramp.tile([gsz, c.DIM], bf16, name=f"part{g}")
                for g, (_, _, gsz, _) in enumerate(rs_groups)
            ]
            rs_g = [
                dramp.tile([gsz // c.NCORES, c.DIM], bf16, name=f"rs{g}")
                for g, (_, _, gsz, _) in enumerate(rs_groups)
            ]

            # ============ Phase 1: projections + RoPE ============
            WG = 8                    # wk/wv k-tiles per DMA
            with tc.tile_pool(name="wpool", bufs=1) as wpool, \
                 tc.tile_pool(name="xpool", bufs=3) as xpool, \
                 tc.tile_pool(name="tabs", bufs=2) as tabs, \
                 tc.tile_pool(name="rope", bufs=2) as ropep, \
                 tc.tile_pool(name="pj_ps", bufs=1, space="PSUM") as pjps:

                # k/v weights ride the gpsimd queue in WG-tile mega-DMAs
                wqb = [None] * (c.NKT // KG)
                wkb, wvb = [], []
                for gi in range(c.NKT // WG):
                    r0 = gi * WG * 2 * P
                    r1 = (gi + 1) * WG * 2 * P
                    wkt = wpool.tile([P, WG, 2, c.HD], fp8, tag="wk",
                                     bufs=c.NKT // WG, name=f"wk_g{gi}")
                    wvt = wpool.tile([P, WG, 2, c.HD], fp8, tag="wv",
                                     bufs=c.NKT // WG, name=f"wv_g{gi}")
                    if gi == 0:
                        # split head: first 2 k-tiles land fast
                        rm = r0 + 2 * 2 * P
                        nc.gpsimd.dma_start(
                            wkt[:, 0:2, :, :],
                            wk8[r0:rm, :].rearrange(
                                "(g i p) h -> p g i h", p=P, i=2))
                        nc.gpsimd.dma_start(
                            wvt[:, 0:2, :, :],
                            wv8[r0:rm, :].rearrange(
                                "(g i p) h -> p g i h", p=P, i=2))
                        nc.gpsimd.dma_start(
                            wkt[:, 2:WG, :, :],
                            wk8[rm:r1, :].rearrange(
                                "(g i p) h -> p g i h", p=P, i=2))
                        nc.gpsimd.dma_start(
                            wvt[:, 2:WG, :, :],
                            wv8[rm:r1, :].rearrange(
                                "(g i p) h -> p g i h", p=P, i=2))
                    else:
                        nc.gpsimd.dma_start(
                            wkt[:],
                            wk8[r0:r1, :].rearrange(
                                "(g i p) h -> p g i h", p=P, i=2))
                        nc.gpsimd.dma_start(
                            wvt[:],
                            wv8[r0:r1, :].rearrange(
                                "(g i p) h -> p g i h", p=P, i=2))
                    wkb.append(wkt)
                    wvb.append(wvt)

                def wk_at(kt):
                    return wkb[kt // WG][:, kt % WG, :, :]     # [P,2,HD] (h,l)

                def wv_at(kt):
                    return wvb[kt // WG][:, kt % WG, :, :]

                def wk_hh(kt):  # [P,2,HD] hi slots of (kt, kt+1)
                    g, l0 = kt // WG, kt % WG
                    return wkb[g][:, l0:l0 + 2, 0, :]

                def wv_hh(kt):
                    g, l0 = kt // WG, kt % WG
                    return wvb[g][:, l0:l0 + 2, 0, :]

                def load_wq(kg):
                    wqt = wpool.tile([P, KG, 2, c.QF], fp8, tag="wq",
                                     bufs=c.NKT // KG, name=f"wq_g{kg}")
                    r0 = kg * KG * 2 * P
                    if kg == 0:
                        rm = r0 + 2 * P
                        nc.sync.dma_start(
                            wqt[:, 0:1, :, :],
                            wq8[r0:rm, :].rearrange(
                                "(g i p) f -> p g i f", p=P, i=2))
                        nc.sync.dma_start(
                            wqt[:, 1:KG, :, :],
                            wq8[rm:r0 + KG * 2 * P, :].rearrange(
                                "(g i p) f -> p g i f", p=P, i=2))
                    else:
                        nc.sync.dma_start(
                            wqt[:],
                            wq8[r0:r0 + KG * 2 * P, :].rearrange(
                                "(g i p) f -> p g i f", p=P, i=2))
                    wqb[kg] = wqt

                for ch in range(c.NCH):
                    t0 = ch * c.TCH
                    bch = ch // CPB           # batch of this chunk
                    lt0 = t0 - bch * c.S      # batch-local token offset
                    q_ps = [
                        pjps.tile([P, c.TCH], fp32, tag=f"q{ft}", bufs=1,
                                  name=f"q_ps{ft}")
                        for ft in range(c.HPC)
                    ]
                    k_ps = pjps.tile([P, c.TCH], fp32, tag="k", bufs=1)
                    v_ps = pjps.tile([P, TPP, P], fp32, tag="v", bufs=1)

                    for kg in range(c.NKT // KG):
                        # one DMA brings KG k-tiles x (lo,hi) = 1MB fp8
                        xt = xpool.tile([P, KG, 2, c.TCH], fp8, tag="xt")
                        r0 = kg * KG * 2 * P
                        if ch == 0 and kg == 0:
                            nc.sync.dma_start(
                                xt[:, 0:1, :, :],
                                x8[r0:r0 + 2 * P, t0:t0 + c.TCH].rearrange(
                                    "(g i p) t -> p g i t", p=P, i=2))
                            nc.sync.dma_start(
                                xt[:, 1:KG, :, :],
                                x8[r0 + 2 * P:r0 + KG * 2 * P,
                                   t0:t0 + c.TCH].rearrange(
                                    "(g i p) t -> p g i t", p=P, i=2))
                        else:
                            nc.sync.dma_start(
                                xt[:],
                                x8[r0:r0 + KG * 2 * P,
                                   t0:t0 + c.TCH].rearrange(
                                    "(g i p) t -> p g i t", p=P, i=2))
                        if ch == 0:
                            load_wq(kg)
                        if ch == 0 and kg == 0:
                            nc.sync.dma_start(tril_sb[:], tril[:])
                            nc.sync.dma_start(idb_sb[:], ident[:])
                        wqt = wqb[kg]
                        for a2 in range(0, KG, 2):
                            kt = kg * KG + a2
                            st = kt == 0
                            sp = kt + 1 == c.NKT - 1

                            def mm_kv():
                                # cross(kt) first: at t=0 it only needs the
                                # small split-head DMA (kt alone), while the
                                # hi.hi pair also waits on kt+1
                                nc.tensor.matmul(
                                    k_ps[:], lhsT=wk_at(kt),
                                    rhs=xt[:, a2, :, :],
                                    start=st, stop=False, perf_mode=DR)
                                nc.tensor.matmul(
                                    k_ps[:], lhsT=wk_hh(kt),
                                    rhs=xt[:, a2:a2 + 2, 1, :],
                                    start=False, stop=False, perf_mode=DR)
                                nc.tensor.matmul(
                                    k_ps[:], lhsT=wk_at(kt + 1),
                                    rhs=xt[:, a2 + 1, :, :],
                                    start=False, stop=sp, perf_mode=DR)
                                # v token-major: x is lhsT
                                for sub in range(TPP):
                                    s0 = sub * P
                                    nc.tensor.matmul(
                                        v_ps[:, sub, :],
                                        lhsT=xt[:, a2, :, s0:s0 + P],
                                        rhs=wv_at(kt),
                                        start=(st and sub == 0), stop=False,
                                        perf_mode=DR)
                                    nc.tensor.matmul(
                                        v_ps[:, sub, :],
                                        lhsT=xt[:, a2:a2 + 2, 1, s0:s0 + P],
                                        rhs=wv_hh(kt),
                                        start=False, stop=False,
                                        perf_mode=DR)
                                    nc.tensor.matmul(
                                        v_ps[:, sub, :],
                                        lhsT=xt[:, a2 + 1, :, s0:s0 + P],
                                        rhs=wv_at(kt + 1),
                                        start=False, stop=sp,
                                        perf_mode=DR)

                            if ch == 0:
                                mm_kv()
                            for ft in range(c.HPC):
                                f0 = ft * P
                                nc.tensor.matmul(
                                    q_ps[ft][:],
                                    lhsT=wqt[:, a2, :, f0:f0 + P],
                                    rhs=xt[:, a2, :, :],
                                    start=st, stop=False, perf_mode=DR)
                                nc.tensor.matmul(
                                    q_ps[ft][:],
                                    lhsT=wqt[:, a2:a2 + 2, 0, f0:f0 + P],
                                    rhs=xt[:, a2:a2 + 2, 1, :],
                                    start=False, stop=False, perf_mode=DR)
                                nc.tensor.matmul(
                                    q_ps[ft][:],
                                    lhsT=wqt[:, a2 + 1, :, f0:f0 + P],
                                    rhs=xt[:, a2 + 1, :, :],
                                    start=False, stop=sp, perf_mode=DR)
                            if ch != 0:
                                mm_kv()

                    # ---- RoPE on all q tiles + k at once (bf16, DVE 4x) ----
                    ct = tabs.tile([P, c.TCH], bf16, tag="cos")
                    st_t = tabs.tile([P, c.TCH], bf16, tag="sin")
                    nc.sync.dma_start(ct[:], cosi[:, t0:t0 + c.TCH])
                    nc.sync.dma_start(st_t[:], sini[:, t0:t0 + c.TCH])

                    qbig = ropep.tile([P, NQT, c.TCH], bf16, tag="qbig",
                                      name="qbig")
                    gt0 = lt0 // P
                    nc.scalar.activation(qbig[:, 0, :], q_ps[0][:], Copy)
                    nc.vector.tensor_copy(qbig[:, 1, :], q_ps[1][:])
                    nc.scalar.activation(qbig[:, 2, :], q_ps[2][:], Copy)
                    nc.vector.tensor_copy(qbig[:, 3, :], q_ps[3][:])
                    nc.scalar.activation(qbig[:, c.HPC, :], k_ps[:], Copy)
                    nc.scalar.activation(v_b[bch][:, gt0, 0:c.HD],
                                         v_ps[:, 0, :], Copy)
                    nc.vector.tensor_copy(v_b[bch][:, gt0 + 1, 0:c.HD],
                                          v_ps[:, 1, :])
                    nc.scalar.activation(v_b[bch][:, gt0 + 2, 0:c.HD],
                                         v_ps[:, 2, :], Copy)
                    nc.vector.tensor_copy(v_b[bch][:, gt0 + 3, 0:c.HD],
                                          v_ps[:, 3, :])

                    qsw = ropep.tile([P, NQT, c.TCH], bf16, tag="qsw",
                                     name="qsw")
                    # pair swap == half-partition block swap (even|odd split)
                    nc.sync.dma_start(qsw[0:64, :, :], qbig[64:128, :, :])
                    nc.sync.dma_start(qsw[64:128, :, :], qbig[0:64, :, :])

                    ctb = ct[:, None, :].to_broadcast((P, NQT, c.TCH))
                    stb = st_t[:, None, :].to_broadcast((P, NQT, c.TCH))
                    eng = nc.vector
                    eng.tensor_mul(qbig[:], qbig[:], ctb)
                    eng.tensor_mul(qsw[:], qsw[:], stb)
                    rr = ropep.tile([P, NQT, c.TCH], bf16, tag="rr", name="rr")
                    eng.tensor_add(rr[:], qbig[:], qsw[:])
                    for ft in range(c.HPC):
                        eng.tensor_copy(
                            qT_b[bch][:, ft, lt0:lt0 + c.TCH], rr[:, ft, :])
                    eng.tensor_copy(
                        kT_b[bch][:, lt0:lt0 + c.TCH], rr[:, c.HPC, :])

            # ====== Phases 2+4 per batch: attention -> wo -> RS ======
            if phases >= 2:
                with tc.tile_pool(name="spool", bufs=3, space="PSUM") as spool, \
                     tc.tile_pool(name="opool", bufs=4, space="PSUM") as opool, \
                     tc.tile_pool(name="tpool", bufs=1, space="PSUM") as tpool, \
                     tc.tile_pool(name="ppool", bufs=32) as ppool, \
                     tc.tile_pool(name="apool", bufs=4) as apool, \
                     tc.tile_pool(name="wop", bufs=1) as wop, \
                     tc.tile_pool(name="atp", bufs=2) as atp, \
                     tc.tile_pool(name="obp", bufs=2) as obp:

                    # wo row-shard as one [P, HPC, 2(lo,hi), DIM] fp8 tile
                    wo_sb = wop.tile([P, c.HPC, 2, c.DIM], fp8, name="wo_sb")
                    for f in range(c.HPC):
                        nc.gpsimd.dma_start(
                            wo_sb[:, f, :, :],
                            wo8[f * 2 * P:(f + 1) * 2 * P, :].rearrange(
                                "(i p) d -> p i d", p=P))

                    def finalize(pend):
                        # bf16 PE transpose of the normalized attention tile,
                        # then split hi/lo fp8 into the attnT store
                        # (layout [P, ft, (h,l), S])
                        p_attnT, p_h, p_qi, p_ao = pend
                        tp = tpool.tile([P, P], bf16, tag="tp", name="tp")
                        nc.tensor.transpose(tp[:], p_ao[:], idb_sb[:])
                        t0c = p_qi * P
                        hi_sl = p_attnT[:, p_h, 0, t0c:t0c + P]
                        nc.vector.tensor_copy(hi_sl, tp[:])
                        nc.vector.tensor_sub(
                            p_attnT[:, p_h, 1, t0c:t0c + P], tp[:], hi_sl)

                    def emit_last_pv(pend):
                        # deferred final PV steps + normalize (ao = 4*attn)
                        p_ops, p_steps, p_start, p_qi, p_attnT, p_h = pend
                        for i, (p_pt, p_off, p_vt) in enumerate(p_steps):
                            nc.tensor.matmul(
                                p_ops, lhsT=p_pt[:, p_off:p_off + P],
                                rhs=p_vt,
                                start=(p_start and i == 0),
                                stop=(i == len(p_steps) - 1),
                            )
                        rec = apool.tile([P, 1], fp32, tag="rec", name="rec")
                        nc.vector.reciprocal(rec[:], p_ops[:, c.HD:c.VW])
                        ao = apool.tile([P, P], bf16, tag="ao", name="ao")
                        nc.vector.tensor_scalar_mul(
                            ao[:], p_ops[:, 0:c.HD], rec[:])
                        return (p_attnT, p_h, p_qi, ao)

                    pending_o = None
                    pend_pv = None
                    o_pp = opool.tile([P, 2, c.VW], fp32, tag="o", bufs=1,
                                      name="o_pp")
                    n_ki = 0
                    SC = 1024             # score tile width (one exp per tile)

                    def score_tile(b, qh, kj, c0, w, tril_m):
                        s_ps = spool.tile([P, SC], fp32, tag="s",
                                          name="s_ps")
                        for sub in range(0, w, c.TCH):
                            ws = min(c.TCH, w - sub)
                            nc.tensor.matmul(
                                s_ps[:, sub:sub + ws],
                                lhsT=kT_b[b][:, kj * P:(kj + 1) * P],
                                rhs=qh[:, c0 + sub:c0 + sub + ws],
                                start=True, stop=True,
                            )
                        pt = ppool.tile([P, SC], bf16, tag="pt", bufs=28,
                                        name="pt")
                        nc.scalar.activation(pt[:, :w], s_ps[:, :w], Exp,
                                             scale=c.SCALE)
                        if tril_m:
                            nc.vector.tensor_mul(pt[:, 0:P], pt[:, 0:P],
                                                 tril_sb[:])
                        return pt

                    def emit_unit(b, h, ki, attnT, pts):
                        nonlocal pending_o, pend_pv, n_ki
                        qh = qT_b[b][:, h, :]
                        q0 = ki * P
                        nmc = (c.S - q0 + SC - 1) // SC
                        for mc in range(nmc):
                            c0 = q0 + mc * SC
                            pts[(ki, mc)] = score_tile(
                                b, qh, ki, c0, min(SC, c.S - c0), mc == 0)
                        if pend_pv is not None:
                            old_fin = pending_o
                            pending_o = emit_last_pv(pend_pv)
                            pend_pv = None
                            if old_fin is not None:
                                finalize(old_fin)
                        qi = ki
                        o_ps = o_pp[:, n_ki % 2, :]
                        n_ki += 1
                        for kj in range(qi):
                            qoff = (qi - kj) * P
                            mc = qoff // SC
                            off = qoff % SC
                            nc.tensor.matmul(
                                o_ps,
                                lhsT=pts[(kj, mc)][:, off:off + P],
                                rhs=v_b[b][:, kj, :],
                                start=(kj == 0), stop=False,
                            )
                        pend_pv = (o_ps,
                                   [(pts[(qi, 0)], 0, v_b[b][:, qi, :])],
                                   qi == 0, qi, attnT, h)

                    attnTs = [
                        atp.tile([P, c.HPC, 2, c.S], fp8, tag="attnT",
                                 name=f"attnT{bb}")
                        for bb in range(c.B)
                    ]
                    pts_next = {}
                    pts_next2 = {}
                    pts_next3 = {}

                    def p4_batch(b, attnT, extra):
                        # row-parallel wo (hi/lo fp8 DR) + pipelined RS
                        nonlocal pending_o, pend_pv
                        ei = 0
                        gids = [g for g, rg in enumerate(rs_groups)
                                if rg[0] == b]
                        for g in gids:
                            _, lt0g, gsz, or0 = rs_groups[g]
                            for tt in range(gsz // P):  # 128-token tiles
                                lt = lt0g + tt * P
                                ob = obp.tile([P, c.DIM], bf16, tag="ob",
                                              bufs=3, name="ob")
                                for cc in range(c.DIM // SC):
                                    o4 = spool.tile([P, SC], fp32,
                                                    tag="s", name="o4_ps")
                                    for half in range(SC // c.TCH):
                                        hb = half * c.TCH
                                        d0 = cc * SC + hb
                                        # hi.hi over ft pairs, then cross
                                        for fp_ in range(0, c.HPC, 2):
                                            nc.tensor.matmul(
                                                o4[:, hb:hb + c.TCH],
                                                lhsT=attnT[:, fp_:fp_ + 2, 0,
                                                           lt:lt + P],
                                                rhs=wo_sb[:, fp_:fp_ + 2, 1,
                                                          d0:d0 + c.TCH],
                                                start=(fp_ == 0), stop=False,
                                                perf_mode=DR)
                                        for f in range(c.HPC):
                                            nc.tensor.matmul(
                                                o4[:, hb:hb + c.TCH],
                                                lhsT=attnT[:, f, :,
                                                           lt:lt + P],
                                                rhs=wo_sb[:, f, :,
                                                          d0:d0 + c.TCH],
                                                start=False,
                                                stop=(f == c.HPC - 1),
                                                perf_mode=DR)
                                    if pend_pv is not None:
                                        old_fin = pending_o
                                        pending_o = emit_last_pv(pend_pv)
                                        pend_pv = None
                                        if old_fin is not None:
                                            finalize(old_fin)
                                    elif pending_o is not None:
                                        finalize(pending_o)
                                        pending_o = None
                                    dst = ob[:, cc * SC:(cc + 1) * SC]
                                    if cc % 2 == 0:
                                        nc.scalar.activation(dst, o4[:], Copy,
                                                             scale=OSCL)
                                    else:
                                        nc.vector.tensor_scalar_mul(
                                            dst, o4[:], c256_sb[:])
                                nc.gpsimd.dma_start(
                                    part_g[g][tt * P:(tt + 1) * P, :], ob[:]
                                )
                                for _ in range(3):
                                    if ei < len(extra):
                                        extra[ei]()
                                        ei += 1
                            if phases >= 3:
                                nc.gpsimd.collective_compute(
                                    "ReduceScatter",
                                    mybir.AluOpType.add,
                                    replica_groups=[list(range(c.NCORES))],
                                    ins=[part_g[g][:].opt()],
                                    outs=[rs_g[g][:].opt()],
                                )
                                nc.sync.dma_start(
                                    out[or0:or0 + gsz // c.NCORES, :],
                                    rs_g[g][:],
                                )
                        assert ei == len(extra)

                    # batch-0 attention, all 4 heads
                    for h in range(c.HPC):
                        pts = {}
                        for ki in range(c.SQT):
                            emit_unit(0, h, ki, attnTs[0], pts)
                    # batch-1's LAST head runs here (not between the two p4
                    # phases): p4(b1) then follows p4(b0) immediately, so its
                    # partials - and the tail ReduceScatter chain - start
                    # ~29us earlier. Heads 0-2 of batch 1 still ride inside
                    # p4(b0) as extras.
                    pts_h3 = {}
                    for ki in range(c.SQT):
                        emit_unit(1, 3, ki, attnTs[1], pts_h3)
                    extra = [
                        (lambda kk=kk, hh=hh, pp=pp: emit_unit(
                            1, hh, kk, attnTs[1], pp))
                        for hh, pp in ((0, pts_next), (1, pts_next2),
                                       (2, pts_next3))
                        for kk in range(c.SQT)
                    ]
                    p4_batch(0, attnTs[0], extra)
                    p4_batch(1, attnTs[1], [])

        for f_ in reversed(free_stat):
            f_()

    nc.compile()
    return nc


def _hilo(a):
    """e4m3 hi/lo decomposition of a float32 array."""
    hi = a.astype(F8)
    lo = (a - hi.astype(np.float32)).astype(F8)
    return hi, lo


def _host_inputs(c, x, wq, wk, wv, wo):
    """Shard + lay out the inputs for the cores."""
    xT = np.ascontiguousarray(x.reshape(c.T, c.DIM).T)    # [DIM, T] fp32

    # x8 rows (kt, slot(lo,hi), p)
    xh, xl = _hilo(xT)
    x8 = np.empty((c.NKT, 2, P, c.T), F8)
    x8[:, 0] = xl.reshape(c.NKT, P, c.T)
    x8[:, 1] = xh.reshape(c.NKT, P, c.T)
    x8 = np.ascontiguousarray(x8.reshape(c.NKT * 2 * P, c.T))

    # even/odd split permutation within each head (q and k only)
    perm_head = np.concatenate([np.arange(0, c.HD, 2), np.arange(1, c.HD, 2)])

    def permute_heads(w):  # w: [DIM, n*HD]
        nh = w.shape[1] // c.HD
        w = w.reshape(c.DIM, nh, c.HD)[:, :, perm_head]
        return np.ascontiguousarray(w.reshape(c.DIM, nh * c.HD))

    def pack_w(w):  # [DIM, F] fp32 -> rows (kt, slot(hi,lo), p)
        hi, lo = _hilo(64.0 * w)
        o = np.empty((c.NKT, 2, P, w.shape[1]), F8)
        o[:, 0] = hi.reshape(c.NKT, P, -1)
        o[:, 1] = lo.reshape(c.NKT, P, -1)
        return np.ascontiguousarray(o.reshape(c.NKT * 2 * P, w.shape[1]))

    wq_p = pack_w(permute_heads(wq).astype(np.float32))
    wk_p = pack_w(permute_heads(wk).astype(np.float32))
    wv_p = pack_w(wv.astype(np.float32))

    def pack_wo(w):  # [QF, DIM] -> rows (ft, slot(lo,hi), p)
        hi, lo = _hilo(64.0 * w)
        o = np.empty((c.HPC, 2, P, c.DIM), F8)
        o[:, 0] = lo.reshape(c.HPC, P, c.DIM)
        o[:, 1] = hi.reshape(c.HPC, P, c.DIM)
        return np.ascontiguousarray(o.reshape(c.HPC * 2 * P, c.DIM))

    # rope tables with 1/64 folded (undoes the 64x weight scale), bf16
    hh = c.HD // 2
    inv = 1.0 / (c.THETA ** (np.arange(0, c.HD, 2, dtype=np.float64) / c.HD))
    pos = (np.arange(c.T) % c.S).astype(np.float64)
    ang = inv[:, None] * pos[None, :]              # [64, T]
    cosv = (np.cos(ang) / 64.0).astype(np.float32)
    sinv = (np.sin(ang) / 64.0).astype(np.float32)
    cosi = np.concatenate([cosv, cosv], 0).astype(BF16)
    sini = np.concatenate([-sinv, sinv], 0).astype(BF16)
    assert hh * 2 == P

    trilm = np.ascontiguousarray(
        np.tril(np.ones((P, P), np.float32)).T
    ).astype(BF16)                                  # [k, q]: 1 iff k<=q
    identm = np.eye(P, dtype=np.float32).astype(BF16)

    KHC = c.KVH // c.NCORES  # kv heads per core (=1)
    in_maps = []
    for cc in range(c.NCORES):
        in_maps.append({
            "x8": x8,
            "wq8": np.ascontiguousarray(wq_p[:, cc * c.QF:(cc + 1) * c.QF]),
            "wk8": np.ascontiguousarray(
                wk_p[:, cc * KHC * c.HD:(cc * KHC + 1) * c.HD]),
            "wv8": np.ascontiguousarray(
                wv_p[:, cc * KHC * c.HD:(cc * KHC + 1) * c.HD]),
            "wo8": pack_wo(
                wo[cc * c.QF:(cc + 1) * c.QF, :].astype(np.float32)),
            "cosi": cosi,
            "sini": sini,
            "tril": trilm,
            "ident": identm,
        })
    return in_maps


def assemble(c, outs):
    """outs[r]: per-core token shards, rows grouped by RS group; within
    group (b, lt0, gsz) core r holds global tokens
    [b*S + lt0 + r*gsz/NC, +gsz/NC)."""
    outs = [np.asarray(o) for o in outs]
    full = np.empty((c.T, c.DIM), np.float32)
    orow = 0
    for b in range(c.B):
        lt = 0
        for gsz in c.RSG[b]:
            sh = gsz // c.NCORES
            for r in range(c.NCORES):
                t0 = b * c.S + lt + r * sh
                full[t0:t0 + sh] = outs[r][orow:orow + sh].astype(np.float32)
            lt += gsz
            orow += sh
    return full.reshape(c.B, c.S, c.DIM)


def kernel(x, wq, wk, wv, wo):
    from concourse import bass_utils

    if "nc" not in _CACHE:
        _CACHE["cfg"] = make_cfg()
        _CACHE["nc"] = _build_graph(_CACHE["cfg"])
    nc = _CACHE["nc"]
    c = _CACHE["cfg"]

    in_maps = _host_inputs(
        c, np.asarray(x), np.asarray(wq), np.asarray(wk),
        np.asarray(wv), np.asarray(wo),
    )
    res = bass_utils.run_bass_kernel_spmd(
        nc, in_maps, core_ids=list(range(c.NCORES)), trace=_TRACE
    )
    _CACHE["last_results"] = res
    outs = [res.results[i]["out"] for i in range(c.NCORES)]
    return assemble(c, outs)
